# revision 2
# baseline (speedup 1.0000x reference)
"""Trainium2 Bass kernel for the 2-layer grid-GCN + linear head.

Math: the GCN aggregation over the fixed graph is a linear operator on
the node axis: out = A @ h per batch column, where
A[j, i] = sum_{edges (i->j)} dinv[i]*dinv[j].  For the 26x26 grid with
row-major node order A is banded (|i-j| <= 26), so with 128-row node
tiles it is block-tridiagonal.  The whole network becomes

    h1 = relu(B1 @ xT + b1)      B1 = w1 * A   (bf16 stationaries)
    h2 = relu(B2 @ h1 + b2)      B2 = w2 * A
    y  = relu(linw.T @ h2 + lin_b)

computed per 512-wide batch-column chunk on the tensor engine, with
ScalarE/VectorE draining PSUM through the relu + bf16 cast.  Batch is
sharded across the 8 NeuronCores (pure data parallel); x is transposed
and cast to bf16 on the host so every DMA is a clean 2D pattern.
"""

import sys

if "/opt/trn_rl_repo" not in sys.path:
    sys.path.insert(0, "/opt/trn_rl_repo")

import numpy as np
import ml_dtypes

N_CORES = 8
N = 676           # nodes (26x26 grid)
B_TOTAL = 65536
COLS = B_TOTAL // N_CORES      # batch columns per core
CHUNK = 512                    # matmul free dim / PSUM bank
GROUP = 2048                   # DMA column-group
N_CHUNKS = COLS // CHUNK
N_GROUPS = COLS // GROUP
N_TILES = (N + 127) // 128     # 6 node tiles
P = [min(128, N - 128 * t) for t in range(N_TILES)]   # [128]*5 + [36]
OFF = [128 * t for t in range(N_TILES)]

bf16 = ml_dtypes.bfloat16

TRACE = False            # test.py flips this to profile
LAST_RESULT = None       # BassKernelResults stash when TRACE


def _neighbors(m):
    return [k for k in (m - 1, m, m + 1) if 0 <= k < N_TILES]


def _pack_blocks(Bmat):
    """Pack block-tridiagonal lhsT blocks into one [128, W] bf16 array.

    Returns (packed, offsets) with offsets[(m, k)] = column offset of the
    [P[k], P[m]] block lhsT = Bmat[k_range, m_range] (out = lhsT.T @ rhs).
    """
    offsets = {}
    w = 0
    for m in range(N_TILES):
        for k in _neighbors(m):
            offsets[(m, k)] = w
            w += P[m]
    packed = np.zeros((128, w), dtype=bf16)
    for m in range(N_TILES):
        for k in _neighbors(m):
            blk = Bmat[OFF[k]:OFF[k] + P[k], OFF[m]:OFF[m] + P[m]]
            packed[: P[k], offsets[(m, k)]:offsets[(m, k)] + P[m]] = blk.astype(bf16)
    return packed, offsets


_PROGRAM_CACHE = {}


def _build_program(wwidth, b1f, b2f, linbf):
    key = (wwidth, b1f, b2f, linbf)
    if key in _PROGRAM_CACHE:
        return _PROGRAM_CACHE[key]

    import concourse.bass as bass
    import concourse.mybir as mybir
    import concourse.tile as tile
    from concourse import bacc

    nc = bacc.Bacc(None, target_bir_lowering=False)
    dt = mybir.dt

    xt_d = nc.dram_tensor("xt", (N, COLS), dt.bfloat16, kind="ExternalInput")
    wc1_d = nc.dram_tensor("wc1", (128, wwidth), dt.bfloat16, kind="ExternalInput")
    wc2_d = nc.dram_tensor("wc2", (128, wwidth), dt.bfloat16, kind="ExternalInput")
    wlin_d = nc.dram_tensor("wlin", (128, N_TILES), dt.bfloat16, kind="ExternalInput")
    y_d = nc.dram_tensor("y", (1, COLS), dt.float32, kind="ExternalOutput")

    with tile.TileContext(nc) as tc:
        with (
            tc.tile_pool(name="weights", bufs=1) as wpool,
            tc.tile_pool(name="xin", bufs=2) as xpool,
            tc.tile_pool(name="acts", bufs=2) as hpool,
            tc.tile_pool(name="yout", bufs=1) as ypool,
            tc.tile_pool(name="ps1", bufs=3, space="PSUM") as ps1pool,
            tc.tile_pool(name="ps2", bufs=3, space="PSUM") as ps2pool,
            tc.tile_pool(name="psl", bufs=2, space="PSUM") as pslpool,
        ):
            wc1 = wpool.tile([128, wwidth], dt.bfloat16, tag="wc1")
            wc2 = wpool.tile([128, wwidth], dt.bfloat16, tag="wc2")
            wlin = wpool.tile([128, N_TILES], dt.bfloat16, tag="wlin")
            nc.sync.dma_start(wc1[:], wc1_d[:])
            nc.sync.dma_start(wc2[:], wc2_d[:])
            nc.sync.dma_start(wlin[:], wlin_d[:])

            y_sb = ypool.tile([1, COLS], dt.float32, tag="y")

            # per-(node-tile, col-group) input tiles, double buffered per tile
            xt_tiles = [[None] * N_GROUPS for _ in range(N_TILES)]

            # recompute block offsets (same logic as _pack_blocks)
            boff = {}
            w = 0
            for m in range(N_TILES):
                for k in _neighbors(m):
                    boff[(m, k)] = w
                    w += P[m]
            assert w == wwidth

            relu = mybir.ActivationFunctionType.Relu

            for c in range(N_CHUNKS):
                g = c // (GROUP // CHUNK)
                if c % (GROUP // CHUNK) == 0:
                    for t in range(N_TILES):
                        xt_tiles[t][g] = xpool.tile(
                            [P[t], GROUP], dt.bfloat16, tag=f"x{t}",
                            name=f"x{t}_{g}",
                        )
                        nc.sync.dma_start(
                            xt_tiles[t][g][:],
                            xt_d[OFF[t]:OFF[t] + P[t],
                                 g * GROUP:(g + 1) * GROUP],
                        )
                cs = slice((c % (GROUP // CHUNK)) * CHUNK,
                           (c % (GROUP // CHUNK) + 1) * CHUNK)

                # ---- conv1: h1 = relu(B1 @ xT + b1) ----
                h1 = []
                for m in range(N_TILES):
                    ps = ps1pool.tile([P[m], CHUNK], dt.float32, tag="ps1")
                    ks = _neighbors(m)
                    for i, k in enumerate(ks):
                        nc.tensor.matmul(
                            ps[:],
                            wc1[: P[k], boff[(m, k)]:boff[(m, k)] + P[m]],
                            xt_tiles[k][g][:, cs],
                            start=(i == 0),
                            stop=(i == len(ks) - 1),
                        )
                    h = hpool.tile([P[m], CHUNK], dt.bfloat16, tag=f"h1_{m}")
                    nc.scalar.activation(h[:], ps[:], relu, bias=b1f)
                    h1.append(h)

                # ---- conv2: h2 = relu(B2 @ h1 + b2) ----
                h2 = []
                for m in range(N_TILES):
                    ps = ps2pool.tile([P[m], CHUNK], dt.float32, tag="ps2")
                    ks = _neighbors(m)
                    for i, k in enumerate(ks):
                        nc.tensor.matmul(
                            ps[:],
                            wc2[: P[k], boff[(m, k)]:boff[(m, k)] + P[m]],
                            h1[k][:],
                            start=(i == 0),
                            stop=(i == len(ks) - 1),
                        )
                    h = hpool.tile([P[m], CHUNK], dt.bfloat16, tag=f"h2_{m}")
                    if b2f == 0.0:
                        nc.vector.tensor_scalar_max(h[:], ps[:], 0.0)
                    else:
                        nc.vector.tensor_scalar(
                            h[:], ps[:], b2f, 0.0,
                            mybir.AluOpType.add, mybir.AluOpType.max,
                        )
                    h2.append(h)

                # ---- linear head: y = relu(linw.T @ h2 + lin_b) ----
                psl = pslpool.tile([1, CHUNK], dt.float32, tag="psl")
                for k in range(N_TILES):
                    nc.tensor.matmul(
                        psl[:],
                        wlin[: P[k], k:k + 1],
                        h2[k][:],
                        start=(k == 0),
                        stop=(k == N_TILES - 1),
                    )
                nc.scalar.activation(
                    y_sb[0:1, c * CHUNK:(c + 1) * CHUNK], psl[:], relu, bias=linbf
                )

            nc.sync.dma_start(y_d[:], y_sb[:])

    nc.compile()
    _PROGRAM_CACHE[key] = nc
    return nc


def kernel(x, w1, b1, w2, b2, lin_w, lin_b, edge_src, edge_dst):
    global LAST_RESULT
    from concourse import bass_utils

    x = np.asarray(x)
    # Build the dense normalized aggregation operator from the edge lists.
    deg = np.zeros(N, np.float64)
    np.add.at(deg, np.asarray(edge_dst), 1.0)
    dinv = 1.0 / np.sqrt(deg)
    normv = dinv[np.asarray(edge_src)] * dinv[np.asarray(edge_dst)]
    A = np.zeros((N, N), np.float64)
    np.add.at(A, (np.asarray(edge_dst), np.asarray(edge_src)), normv)

    w1f = float(np.asarray(w1).reshape(-1)[0])
    w2f = float(np.asarray(w2).reshape(-1)[0])
    b1f = float(np.asarray(b1).reshape(-1)[0])
    b2f = float(np.asarray(b2).reshape(-1)[0])
    linbf = float(np.asarray(lin_b).reshape(-1)[0])

    wc1_np, _ = _pack_blocks((w1f * A).astype(np.float32))
    wc2_np, _ = _pack_blocks((w2f * A).astype(np.float32))
    wlin_np = np.zeros((128, N_TILES), dtype=bf16)
    lw = np.asarray(lin_w).reshape(-1)
    for t in range(N_TILES):
        wlin_np[: P[t], t] = lw[OFF[t]:OFF[t] + P[t]].astype(bf16)

    nc = _build_program(wc1_np.shape[1], b1f, b2f, linbf)

    # host-side: transpose, cast, shard along batch
    xt = np.ascontiguousarray(x.T).astype(bf16)        # [676, 65536]
    in_maps = []
    for c in range(N_CORES):
        in_maps.append({
            "xt": np.ascontiguousarray(xt[:, c * COLS:(c + 1) * COLS]),
            "wc1": wc1_np,
            "wc2": wc2_np,
            "wlin": wlin_np,
        })

    res = bass_utils.run_bass_kernel_spmd(
        nc, in_maps, list(range(N_CORES)), trace=TRACE
    )
    if TRACE:
        LAST_RESULT = res
    out = np.concatenate([res.results[c]["y"].reshape(-1) for c in range(N_CORES)])
    return out.reshape(B_TOTAL, 1).astype(np.float32)


# revision 16
# speedup vs baseline: 1.0114x; 1.0114x over previous
"""Trainium2 Bass kernel for the 2-layer grid-GCN + linear head.

Math: the GCN aggregation over the fixed graph is a linear operator on
the node axis: out = A @ h per batch column, where
A[j, i] = sum_{edges (i->j)} dinv[i]*dinv[j].  For the 26x26 grid with
row-major node order A is banded (|i-j| <= 26), so with 128-row node
tiles it is block-tridiagonal.  The whole network becomes

    h1 = relu(B1 @ xT + b1)      B1 = w1 * A   (bf16 stationaries)
    h2 = relu(B2 @ h1 + b2)      B2 = w2 * A
    y  = relu(linw.T @ h2 + lin_b)

computed per 512-wide batch-column chunk on the tensor engine, with
ScalarE (conv1 + head) and VectorE (conv2) draining PSUM through the
relu + bf16 cast.  Batch is sharded across the 8 NeuronCores (pure data
parallel); x is transposed and cast to bf16 on the host so every DMA is
a clean 2D pattern.
"""

import sys

if "/opt/trn_rl_repo" not in sys.path:
    sys.path.insert(0, "/opt/trn_rl_repo")

import numpy as np
import ml_dtypes

N_CORES = 8
N = 676           # nodes (26x26 grid)
B_TOTAL = 65536
COLS = B_TOTAL // N_CORES      # batch columns per core
CHUNK = 512                    # matmul free dim / PSUM bank
GROUP = 2048                   # DMA column-group
N_CHUNKS = COLS // CHUNK
N_GROUPS = COLS // GROUP
N_TILES = (N + 127) // 128     # 6 node tiles
P = [min(128, N - 128 * t) for t in range(N_TILES)]   # [128]*5 + [36]
OFF = [128 * t for t in range(N_TILES)]

bf16 = ml_dtypes.bfloat16

TRACE = False            # test.py flips this to profile
LAST_RESULT = None       # BassKernelResults stash when TRACE


def _neighbors(m):
    return [k for k in (m - 1, m, m + 1) if 0 <= k < N_TILES]


_BOFF = {}
_W = 0
for _m in range(N_TILES):
    for _k in _neighbors(_m):
        _BOFF[(_m, _k)] = _W
        _W += P[_m]


def _pack_blocks(Bmat):
    """Pack block-tridiagonal lhsT blocks into one [128, W] bf16 array.

    Block (m, k) = Bmat[tile k rows, tile m cols] at columns _BOFF[(m, k)]
    (lhsT layout: out = lhsT.T @ rhs contracts the partition dim k).
    """
    packed = np.zeros((128, _W), dtype=bf16)
    for m in range(N_TILES):
        for k in _neighbors(m):
            blk = Bmat[OFF[k]:OFF[k] + P[k], OFF[m]:OFF[m] + P[m]]
            packed[: P[k], _BOFF[(m, k)]:_BOFF[(m, k)] + P[m]] = blk.astype(bf16)
    return packed


_PROGRAM_CACHE = {}


def _build_program(b1f, b2f, linbf):
    key = (b1f, b2f, linbf)
    if key in _PROGRAM_CACHE:
        return _PROGRAM_CACHE[key]

    import concourse.mybir as mybir
    import concourse.tile as tile
    from concourse import bacc

    nc = bacc.Bacc(None, target_bir_lowering=False)
    dt = mybir.dt

    xt_d = nc.dram_tensor("xt", (N, COLS), dt.bfloat16, kind="ExternalInput")
    wc1_d = nc.dram_tensor("wc1", (128, _W), dt.bfloat16, kind="ExternalInput")
    wc2_d = nc.dram_tensor("wc2", (128, _W), dt.bfloat16, kind="ExternalInput")
    wlin_d = nc.dram_tensor("wlin", (128, N_TILES), dt.bfloat16, kind="ExternalInput")
    y_d = nc.dram_tensor("y", (1, COLS), dt.float32, kind="ExternalOutput")

    with tile.TileContext(nc) as tc:
        with (
            tc.tile_pool(name="weights", bufs=1) as wpool,
            tc.tile_pool(name="xin", bufs=2) as xpool,
            tc.tile_pool(name="acts", bufs=2) as hpool,
            tc.tile_pool(name="yout", bufs=1) as ypool,
            tc.tile_pool(name="ps1", bufs=3, space="PSUM") as ps1pool,
            tc.tile_pool(name="ps2", bufs=3, space="PSUM") as ps2pool,
            tc.tile_pool(name="psl", bufs=2, space="PSUM") as pslpool,
        ):
            # x chunk 0 first so compute starts ASAP, then weights, then rest
            xt_tiles = [[None] * N_GROUPS for _ in range(N_TILES)]
            for t in range(N_TILES):
                xt_tiles[t][0] = xpool.tile([P[t], GROUP], dt.bfloat16,
                                            tag=f"x{t}", name=f"x{t}_0")
                nc.sync.dma_start(
                    xt_tiles[t][0][:, 0:CHUNK],
                    xt_d[OFF[t]:OFF[t] + P[t], 0:CHUNK],
                )

            wc1 = wpool.tile([128, _W], dt.bfloat16, tag="wc1")
            wc2 = wpool.tile([128, _W], dt.bfloat16, tag="wc2")
            wlin = wpool.tile([128, N_TILES], dt.bfloat16, tag="wlin")
            nc.sync.dma_start(wc1[:], wc1_d[:])
            nc.sync.dma_start(wc2[:], wc2_d[:])
            nc.sync.dma_start(wlin[:], wlin_d[:])

            for t in range(N_TILES):
                nc.sync.dma_start(
                    xt_tiles[t][0][:, CHUNK:GROUP],
                    xt_d[OFF[t]:OFF[t] + P[t], CHUNK:GROUP],
                )

            y_sb = ypool.tile([1, COLS], dt.float32, tag="y")
            relu = mybir.ActivationFunctionType.Relu

            for c in range(N_CHUNKS):
                g = c // (GROUP // CHUNK)
                if c % (GROUP // CHUNK) == 0 and g > 0:
                    for t in range(N_TILES):
                        xt_tiles[t][g] = xpool.tile(
                            [P[t], GROUP], dt.bfloat16, tag=f"x{t}",
                            name=f"x{t}_{g}",
                        )
                        nc.sync.dma_start(
                            xt_tiles[t][g][:],
                            xt_d[OFF[t]:OFF[t] + P[t],
                                 g * GROUP:(g + 1) * GROUP],
                        )
                cs = slice((c % (GROUP // CHUNK)) * CHUNK,
                           (c % (GROUP // CHUNK) + 1) * CHUNK)

                # ---- conv1: h1 = relu(B1 @ xT + b1) ----
                h1 = []
                for m in range(N_TILES):
                    ps = ps1pool.tile([P[m], CHUNK], dt.float32, tag="ps1",
                                      name=f"ps1_{m}")
                    ks = _neighbors(m)
                    for i, k in enumerate(ks):
                        nc.tensor.matmul(
                            ps[:],
                            wc1[: P[k], _BOFF[(m, k)]:_BOFF[(m, k)] + P[m]],
                            xt_tiles[k][g][:, cs],
                            start=(i == 0),
                            stop=(i == len(ks) - 1),
                        )
                    h = hpool.tile([P[m], CHUNK], dt.bfloat16,
                                   tag=f"h1_{m}", name=f"h1_{m}")
                    nc.scalar.activation(h[:], ps[:], relu, bias=b1f)
                    h1.append(h)

                # ---- conv2: h2 = relu(B2 @ h1 + b2) ----
                h2 = []
                for m in range(N_TILES):
                    ps = ps2pool.tile([P[m], CHUNK], dt.float32, tag="ps2",
                                      name=f"ps2_{m}")
                    ks = _neighbors(m)
                    for i, k in enumerate(ks):
                        nc.tensor.matmul(
                            ps[:],
                            wc2[: P[k], _BOFF[(m, k)]:_BOFF[(m, k)] + P[m]],
                            h1[k][:],
                            start=(i == 0),
                            stop=(i == len(ks) - 1),
                        )
                    h = hpool.tile([P[m], CHUNK], dt.bfloat16,
                                   tag=f"h2_{m}", name=f"h2_{m}")
                    if b2f == 0.0:
                        nc.vector.tensor_scalar_max(h[:], ps[:], 0.0)
                    else:
                        nc.vector.tensor_scalar(
                            h[:], ps[:], b2f, 0.0,
                            mybir.AluOpType.add, mybir.AluOpType.max,
                        )
                    h2.append(h)

                # ---- linear head: y = relu(linw.T @ h2 + lin_b) ----
                psl = pslpool.tile([1, CHUNK], dt.float32, tag="psl",
                                   name="psl")
                for k in range(N_TILES):
                    nc.tensor.matmul(
                        psl[:],
                        wlin[: P[k], k:k + 1],
                        h2[k][:],
                        start=(k == 0),
                        stop=(k == N_TILES - 1),
                    )
                nc.scalar.activation(
                    y_sb[0:1, c * CHUNK:(c + 1) * CHUNK], psl[:], relu,
                    bias=linbf,
                )

            nc.sync.dma_start(y_d[:], y_sb[:])

    nc.compile()
    _PROGRAM_CACHE[key] = nc
    return nc


def kernel(x, w1, b1, w2, b2, lin_w, lin_b, edge_src, edge_dst):
    global LAST_RESULT
    from concourse import bass_utils

    x = np.asarray(x)
    # Build the dense normalized aggregation operator from the edge lists.
    deg = np.zeros(N, np.float64)
    np.add.at(deg, np.asarray(edge_dst), 1.0)
    dinv = 1.0 / np.sqrt(deg)
    normv = dinv[np.asarray(edge_src)] * dinv[np.asarray(edge_dst)]
    A = np.zeros((N, N), np.float64)
    np.add.at(A, (np.asarray(edge_dst), np.asarray(edge_src)), normv)

    w1f = float(np.asarray(w1).reshape(-1)[0])
    w2f = float(np.asarray(w2).reshape(-1)[0])
    b1f = float(np.asarray(b1).reshape(-1)[0])
    b2f = float(np.asarray(b2).reshape(-1)[0])
    linbf = float(np.asarray(lin_b).reshape(-1)[0])

    wc1_np = _pack_blocks((w1f * A).astype(np.float32))
    wc2_np = _pack_blocks((w2f * A).astype(np.float32))
    wlin_np = np.zeros((128, N_TILES), dtype=bf16)
    lw = np.asarray(lin_w).reshape(-1)
    for t in range(N_TILES):
        wlin_np[: P[t], t] = lw[OFF[t]:OFF[t] + P[t]].astype(bf16)

    nc = _build_program(b1f, b2f, linbf)

    # host-side: transpose, cast, shard along batch
    xt = np.ascontiguousarray(x.T).astype(bf16)        # [676, 65536]
    in_maps = []
    for c in range(N_CORES):
        in_maps.append({
            "xt": np.ascontiguousarray(xt[:, c * COLS:(c + 1) * COLS]),
            "wc1": wc1_np,
            "wc2": wc2_np,
            "wlin": wlin_np,
        })

    res = bass_utils.run_bass_kernel_spmd(
        nc, in_maps, list(range(N_CORES)), trace=TRACE
    )
    if TRACE:
        LAST_RESULT = res
    out = np.concatenate([res.results[c]["y"].reshape(-1) for c in range(N_CORES)])
    return out.reshape(B_TOTAL, 1).astype(np.float32)
